# revision 1
# baseline (speedup 1.0000x reference)
"""BilinearInteraction (field_interaction) on 8 TRN2 NeuronCores.

  out[b,f,d] = emb[b,f,d] * sum_{g!=f, e} W[f,g,d,e] * emb[b,g,e]

Strategy (data-parallel, per sharding hint):
  - Host: fold the f!=g mask into W and permute it to a single GEMM matrix
    w2[g*32+e, f*32+d]; flatten embeddings to x[B, 1280]; shard batch over
    8 cores (2048 rows each); replicate w2.
  - Device (per core): out = x * (x @ w2), computed as f32r (full-rate fp32)
    matmuls on the TensorEngine. x row-blocks are transposed on-chip with
    PE transpose-mode matmuls (packed 4-per-PSUM-bank, drained to SBUF by
    the Scalar engine), w2 stays SBUF-resident, DVE does the final
    elementwise multiply, with transposes of batch-tile bt+1 software-
    pipelined into the matmul stream of batch-tile bt.
"""

from contextlib import ExitStack

import numpy as np

BATCH = 16384
NUM_FIELDS = 40
EMBED_DIM = 32
N_CORES = 8

B_LOCAL = BATCH // N_CORES   # 2048
K = NUM_FIELDS * EMBED_DIM   # 1280
P = 128
NBT = B_LOCAL // P           # 16
NKT = K // P                 # 10
N_CHUNKS = [(0, 512), (512, 512), (1024, 256)]
TR_GROUPS = [(0, 4), (4, 4), (8, 2)]

_NC_CACHE = {}


def _build_kernel():
    import concourse.bacc as bacc
    import concourse.mybir as mybir
    import concourse.tile as tile

    F32 = mybir.dt.float32
    F32R = mybir.dt.float32r
    N = K

    nc = bacc.Bacc("TRN2", target_bir_lowering=False, debug=False,
                   num_devices=N_CORES)

    x_d = nc.declare_dram_parameter("x", [B_LOCAL, K], F32, isOutput=False)
    w_d = nc.declare_dram_parameter("w2", [K, N], F32, isOutput=False)
    i_d = nc.declare_dram_parameter("ident", [P, P], F32, isOutput=False)
    o_d = nc.declare_dram_parameter("out", [B_LOCAL, N], F32, isOutput=True)

    with tile.TileContext(nc) as tc, ExitStack() as ctx:
        wpool = ctx.enter_context(tc.tile_pool(name="w", bufs=1))
        cpool = ctx.enter_context(tc.tile_pool(name="const", bufs=1))
        xpool = ctx.enter_context(tc.tile_pool(name="x", bufs=4))
        xtpool = ctx.enter_context(tc.tile_pool(name="xt", bufs=3))
        opool = ctx.enter_context(tc.tile_pool(name="o", bufs=4))
        trps = ctx.enter_context(tc.tile_pool(name="trps", bufs=2, space="PSUM"))
        accps = ctx.enter_context(tc.tile_pool(name="accps", bufs=2, space="PSUM"))

        ident = cpool.tile([P, P], F32R)
        nc.sync.dma_start(ident[:], i_d[:].bitcast(F32R))

        x_tiles = {}
        x_tiles[0] = xpool.tile([P, K], F32R, name="x_sb0", tag="x_sb")
        for kt in range(NKT):
            nc.sync.dma_start(x_tiles[0][:, kt * P:(kt + 1) * P],
                              x_d[0:P, kt * P:(kt + 1) * P].bitcast(F32R))

        w_sb = []
        for kt in range(NKT):
            wt = wpool.tile([P, N], F32R, name=f"w{kt}", tag=f"w{kt}")
            nc.sync.dma_start(wt[:], w_d[kt * P:(kt + 1) * P, :].bitcast(F32R))
            w_sb.append(wt)

        xt_tiles = {}

        def emit_tr_group(bt, g):
            kt0, cnt = TR_GROUPS[g]
            x_sb = x_tiles[bt]
            tr = trps.tile([P, 512], F32R, name=f"tr{bt}_{g}", tag="tr")
            for i in range(cnt):
                kt = kt0 + i
                nc.tensor.transpose(tr[:, i * P:(i + 1) * P],
                                    x_sb[:, kt * P:(kt + 1) * P], ident[:])
            nc.scalar.copy(xt_tiles[bt][:, kt0 * P:(kt0 + cnt) * P],
                           tr[:, 0:cnt * P])

        xt_tiles[0] = xtpool.tile([P, K], F32R, name="xt0", tag="xt")
        for g in range(3):
            emit_tr_group(0, g)

        for bt in range(NBT):
            x_sb = x_tiles[bt]
            acc = [accps.tile([P, sz], F32, name=f"acc{j}_{bt}", tag=f"acc{j}")
                   for j, (_, sz) in enumerate(N_CHUNKS)]

            for kt in range(NKT):
                lhsT = xt_tiles[bt][:, kt * P:(kt + 1) * P]
                for j, (n0, sz) in enumerate(N_CHUNKS):
                    nc.tensor.matmul(acc[j][:], lhsT, w_sb[kt][:, n0:n0 + sz],
                                     start=(kt == 0), stop=(kt == NKT - 1))
                if bt + 1 < NBT:
                    if kt == 3:
                        x_tiles[bt + 1] = xpool.tile([P, K], F32R,
                                                     name=f"x_sb{bt+1}",
                                                     tag="x_sb")
                        nc.sync.dma_start(
                            x_tiles[bt + 1][:],
                            x_d[(bt + 1) * P:(bt + 2) * P, :].bitcast(F32R))
                        xt_tiles[bt + 1] = xtpool.tile([P, K], F32R,
                                                       name=f"xt{bt+1}",
                                                       tag="xt")
                    elif kt in (6, 7, 8):
                        emit_tr_group(bt + 1, kt - 6)

            o_sb = opool.tile([P, N], F32)
            for j, (n0, sz) in enumerate(N_CHUNKS):
                nc.vector.tensor_mul(o_sb[:, n0:n0 + sz], x_sb[:, n0:n0 + sz],
                                     acc[j][:])
            nc.sync.dma_start(o_d[bt * P:(bt + 1) * P, :], o_sb[:])
            if bt > 0:
                del x_tiles[bt - 1], xt_tiles[bt - 1]

    nc.compile()
    return nc


def _get_nc():
    if "nc" not in _NC_CACHE:
        _NC_CACHE["nc"] = _build_kernel()
    return _NC_CACHE["nc"]


def kernel(embeddings: np.ndarray, bilinear_W: np.ndarray) -> np.ndarray:
    from concourse.bass_utils import run_bass_kernel_spmd

    embeddings = np.ascontiguousarray(np.asarray(embeddings, dtype=np.float32))
    bilinear_W = np.ascontiguousarray(np.asarray(bilinear_W, dtype=np.float32))
    F, D = NUM_FIELDS, EMBED_DIM

    # fold the f!=g mask into W and permute to the GEMM layout:
    # w2[g*D+e, f*D+d] = W[f,g,d,e] * (f != g)
    mask = (1.0 - np.eye(F, dtype=np.float32))[:, :, None, None]
    w2 = np.ascontiguousarray(
        (bilinear_W * mask).transpose(1, 3, 0, 2).reshape(F * D, F * D))

    x = embeddings.reshape(BATCH, F * D)
    shards = np.split(x, N_CORES, axis=0)
    ident = np.eye(P, dtype=np.float32)
    in_maps = [{"x": np.ascontiguousarray(s), "w2": w2, "ident": ident}
               for s in shards]

    nc = _get_nc()
    res = run_bass_kernel_spmd(nc, in_maps, list(range(N_CORES)))
    out = np.concatenate([res.results[i]["out"] for i in range(N_CORES)],
                         axis=0)
    return out.reshape(BATCH, F, D).astype(np.float32, copy=False)


# revision 3
# speedup vs baseline: 1.0349x; 1.0349x over previous
"""BilinearInteraction (field_interaction) on 8 TRN2 NeuronCores.

  out[b,f,d] = emb[b,f,d] * sum_{g!=f, e} W[f,g,d,e] * emb[b,g,e]

Strategy (data-parallel, per sharding hint):
  - Host: fold the f!=g mask into W and permute it to a single GEMM matrix
    w2[g*32+e, f*32+d]; flatten embeddings to x[B, 1280]; shard batch over
    8 cores (2048 rows each); replicate w2.
  - Precision: x and w2 ship as fp16 (halves HBM traffic; PSUM accumulates
    fp32; rel err ~5e-4, far inside the gate).
  - Device (per core): out = x * (x @ w2): fp16 TensorEngine matmuls at
    1 cyc/row. x row-blocks are transposed on-chip with PE transpose-mode
    matmuls whose 16-bit payloads are *labeled* bf16 (fp16 transpose-mode
    is broken on TRN2 silicon; x1.0-in-bf16 is bit-exact), packed
    4-per-PSUM-bank and drained to SBUF by the otherwise-idle Scalar
    engine, then bitcast back to fp16 as the matmul stationary operand.
    w2 stays SBUF-resident in per-(k,n)-chunk tiles so the DMA-paced ramp
    has fine-grained dependencies; transposes of batch-tile bt+1 are
    software-pipelined into the matmul stream of batch-tile bt; DVE does
    the final elementwise multiply out of PSUM.
"""

from contextlib import ExitStack

import numpy as np

BATCH = 16384
NUM_FIELDS = 40
EMBED_DIM = 32
N_CORES = 8

B_LOCAL = BATCH // N_CORES   # 2048
K = NUM_FIELDS * EMBED_DIM   # 1280
P = 128
NBT = B_LOCAL // P           # 16
NKT = K // P                 # 10
N_CHUNKS = [(0, 512), (512, 512), (1024, 256)]
TR_GROUPS = [(0, 4), (4, 4), (8, 2)]

_NC_CACHE = {}


def _build_kernel():
    import concourse.bacc as bacc
    import concourse.mybir as mybir
    import concourse.tile as tile

    F32 = mybir.dt.float32
    FP16 = mybir.dt.float16
    BF16 = mybir.dt.bfloat16
    N = K

    nc = bacc.Bacc("TRN2", target_bir_lowering=False, debug=False,
                   num_devices=N_CORES)

    x_d = nc.declare_dram_parameter("x", [B_LOCAL, K], FP16, isOutput=False)
    w_d = nc.declare_dram_parameter("w2", [K, N], FP16, isOutput=False)
    i_d = nc.declare_dram_parameter("ident", [P, P], BF16, isOutput=False)
    o_d = nc.declare_dram_parameter("out", [B_LOCAL, N], F32, isOutput=True)

    from contextlib import ExitStack
    with tile.TileContext(nc) as tc, ExitStack() as ctx:
        wpool = ctx.enter_context(tc.tile_pool(name="w", bufs=1))
        cpool = ctx.enter_context(tc.tile_pool(name="const", bufs=1))
        xpool = ctx.enter_context(tc.tile_pool(name="x", bufs=4))
        xtpool = ctx.enter_context(tc.tile_pool(name="xt", bufs=3))
        opool = ctx.enter_context(tc.tile_pool(name="o", bufs=4))
        trps = ctx.enter_context(tc.tile_pool(name="trps", bufs=2, space="PSUM"))
        accps = ctx.enter_context(tc.tile_pool(name="accps", bufs=2, space="PSUM"))

        ident = cpool.tile([P, P], BF16)
        nc.sync.dma_start(ident[:], i_d[:])

        x_tiles = {}

        def load_x(bt):
            parts = []
            for g, (n0, sz) in enumerate(N_CHUNKS):
                t = xpool.tile([P, sz], FP16, name=f"x{bt}_{g}", tag=f"x_g{g}")
                nc.sync.dma_start(t[:], x_d[bt * P:(bt + 1) * P, n0:n0 + sz])
                parts.append(t)
            x_tiles[bt] = parts

        load_x(0)

        w_sb = [[None] * len(N_CHUNKS) for _ in range(NKT)]
        for kt in range(NKT):
            for j, (n0, sz) in enumerate(N_CHUNKS):
                wt = wpool.tile([P, sz], FP16, name=f"w{kt}_{j}", tag=f"w{kt}_{j}")
                nc.sync.dma_start(wt[:], w_d[kt * P:(kt + 1) * P, n0:n0 + sz])
                w_sb[kt][j] = wt

        xt_tiles = {}

        def emit_tr_group(bt, g):
            kt0, cnt = TR_GROUPS[g]
            xg = x_tiles[bt][g]
            tr = trps.tile([P, 512], BF16, name=f"tr{bt}_{g}", tag="tr")
            for i in range(cnt):
                nc.tensor.transpose(tr[:, i * P:(i + 1) * P],
                                    xg[:, i * P:(i + 1) * P].bitcast(BF16),
                                    ident[:])
            nc.scalar.copy(xt_tiles[bt][:, kt0 * P:(kt0 + cnt) * P],
                           tr[:, 0:cnt * P])

        xt_tiles[0] = xtpool.tile([P, K], BF16, name="xt0", tag="xt")
        for g in range(3):
            emit_tr_group(0, g)

        for bt in range(NBT):
            acc = [accps.tile([P, sz], F32, name=f"acc{j}_{bt}", tag=f"acc{j}")
                   for j, (_, sz) in enumerate(N_CHUNKS)]
            last = bt == NBT - 1

            for kt in range(NKT):
                lhsT = xt_tiles[bt][:, kt * P:(kt + 1) * P].bitcast(FP16)
                for j, (n0, sz) in enumerate(N_CHUNKS):
                    nc.tensor.matmul(acc[j][:], lhsT, w_sb[kt][j][:],
                                     start=(kt == 0), stop=(kt == NKT - 1))
                if bt + 1 < NBT:
                    if kt == 3:
                        load_x(bt + 1)
                        xt_tiles[bt + 1] = xtpool.tile([P, K], BF16,
                                                       name=f"xt{bt+1}",
                                                       tag="xt")
                    elif kt in (6, 7, 8):
                        emit_tr_group(bt + 1, kt - 6)

            o_sb = opool.tile([P, N], F32)
            for j, (n0, sz) in enumerate(N_CHUNKS):
                nc.vector.tensor_mul(o_sb[:, n0:n0 + sz], x_tiles[bt][j][:],
                                     acc[j][:])
                if last:
                    nc.sync.dma_start(o_d[bt * P:(bt + 1) * P, n0:n0 + sz],
                                      o_sb[:, n0:n0 + sz])
            if not last:
                nc.sync.dma_start(o_d[bt * P:(bt + 1) * P, :], o_sb[:])
            if bt > 0:
                del x_tiles[bt - 1], xt_tiles[bt - 1]

    nc.compile()
    return nc


def _get_nc():
    if "nc" not in _NC_CACHE:
        _NC_CACHE["nc"] = _build_kernel()
    return _NC_CACHE["nc"]


def kernel(embeddings: np.ndarray, bilinear_W: np.ndarray) -> np.ndarray:
    from concourse.bass_utils import run_bass_kernel_spmd

    embeddings = np.ascontiguousarray(np.asarray(embeddings, dtype=np.float32))
    bilinear_W = np.ascontiguousarray(np.asarray(bilinear_W, dtype=np.float32))
    F, D = NUM_FIELDS, EMBED_DIM

    # fold the f!=g mask into W and permute to the GEMM layout:
    # w2[g*D+e, f*D+d] = W[f,g,d,e] * (f != g)
    mask = (1.0 - np.eye(F, dtype=np.float32))[:, :, None, None]
    w2 = np.ascontiguousarray(
        (bilinear_W * mask).transpose(1, 3, 0, 2).reshape(F * D, F * D))

    import ml_dtypes

    x = embeddings.reshape(BATCH, F * D).astype(np.float16)
    shards = np.split(x, N_CORES, axis=0)
    ident = np.eye(P, dtype=np.float32).astype(ml_dtypes.bfloat16)
    w2_h = w2.astype(np.float16)
    in_maps = [{"x": np.ascontiguousarray(s), "w2": w2_h, "ident": ident}
               for s in shards]

    nc = _get_nc()
    res = run_bass_kernel_spmd(nc, in_maps, list(range(N_CORES)))
    out = np.concatenate([res.results[i]["out"] for i in range(N_CORES)],
                         axis=0)
    return out.reshape(BATCH, F, D).astype(np.float32, copy=False)
